# revision 28
# baseline (speedup 1.0000x reference)
"""MoE (top-2 of 8 experts, SwiGLU) Trainium2 kernel.

Strategy (expert parallelism + overflow-tile load balancing):
  - Host: compute router logits/top-2/softmax, dispatch tokens to
    experts. Every core gets B=4096 primary tokens of its own expert
    (the global mean load is exactly 4096 = 2*16384/8). Each expert's
    tokens beyond B ("overflow", ~256 total here) are chopped into
    chunks of at most R (minimal R with sum_e ceil(o_e/R) <= 8, R=44
    for this routing) and each chunk is shipped to some core's single
    overflow tile together with THAT expert's weight set. This removes
    the load imbalance (max_e n_e = 4213 would otherwise pad every
    core) at the cost of one extra ragged tile and a second streamed
    weight set per core.
  - Device: 8 NeuronCores; core c runs its expert's SwiGLU FFN over 8
    full 512-token tiles, then the R-token overflow tile with the
    second weight set. All matmuls bf16 with fp32 PSUM accumulation.
  - Host: weighted scatter-add combine (two disjoint scatters: primary
    and overflow; each has unique row indices).

Device compute per core (transposed so every matmul uses natural,
transpose-free operand layouts; PSUM accumulates over the contraction):
  hT[h_chunk, tok] = wg.T @ xt   (accumulate K=D over 8 chunks of 128)
  h2 = silu(hT_gate) * hT_up     (ACT silu + DVE mul, bf16 out)
  yT[d_chunk, tok] = wd.T @ h2   (accumulate K=H over 16 chunks of 128)

The steady state runs at the PE roofline cadence (512 cols / 2.4 GHz +
2.5ns NX issue = 216ns per matmul, measured); the overheads around it
are attacked as follows:
  - PE warmup: 14 dummy matmuls on a memset tile run during the DMA ramp
    so the HAM clock-gate reaches 8/8 (2.4 GHz) before real work and the
    cold 1.2 GHz cycles are spent on throwaway work.
  - Tile 0 computes all 16 GATE chains, then all 16 UP chains: the
    up-weight DMA deadline moves from ~16us (undeliverable while the DMA
    rings spin up) to ~42us. Weights stream in exact consumption order.
  - Ring allocation: the ACT ring carries ONLY the xt0 k0..k5 pieces and
    goes idle by ~13us; everything else (wg0 halves, xt0 k6..7, wg1..15,
    wu0..15, wd0..7, xt1) streams on the sync ring in consumption order,
    so the full DMA fabric serves the gate stream during the ramp-stall
    window. Later token tiles prefetch behind it.
  - The overflow weight set streams into the SAME SBUF buffers (pool
    tags) as the primary weights, with each block's DMA gated on that
    block's last primary read; transfers ride the otherwise-idle scalar
    ring from ~600us and land well before the overflow tile needs them.
  - Zero SWDGE transfers: 8 fewer DMA-lane semaphores shrink the
    per-engine NEFF epilogue inside the measured window, and the exit
    skips the redundant semaphore-clear + second barrier.
  - bf16 output (error budget allows it: rel_err 4.7e-3 vs 2e-2 gate).

Weights are host-prepacked to [p=128, out_block, k_chunk, 128] so each
128-column weight block is one contiguous DMA.

NOTE on fp8: DoubleRow fp8 matmuls were prototyped (2x PE rate, error
budget fits a low-routing-weight token subset) but the mere presence of
perf-mode matmuls in a program serializes LDWEIGHTS against MATMUL for
every bf16 instruction (+43ns each, measured 216->259ns), costing more
than the fp8 region saves at any error-feasible split. See
kernel_fp8_attempt.py.
"""

import sys

if "/opt/trn_rl_repo" not in sys.path:
    sys.path.insert(0, "/opt/trn_rl_repo")

import ml_dtypes
import numpy as np

NUM_EXPERTS = 8
TOP_K = 2
EMB = 1024
HID = 2048
P = 128
KD = EMB // P  # 8
KH = HID // P  # 16
TOK = 512  # main token tile (one PSUM bank of f32)
B_PRIMARY = 4096  # per-core primary capacity == mean load (2*T/8)

_BF16 = ml_dtypes.bfloat16


def _make_tile_context(nc):
    """TileContext whose emitted instructions carry at most ONE sem wait.

    The walrus codegen bundled in this environment rejects any instruction
    with more than one sync-wait command ("Too many sync wait commands").
    Tile's scheduler freely attaches several waits to one instruction (and
    its exit drain waits on every frontier semaphore), so hoist all but the
    last wait onto dedicated same-engine NoOps immediately preceding the
    instruction.
    """
    import concourse.mybir as mybir
    import concourse.tile as tile
    from concourse.vector_clock import ScopedClock

    class OneWaitTC(tile.TileContext):
        def _split_waits(self, inst):
            si = getattr(inst, "sync_info", None)
            if si is None or not si.on_wait or len(si.on_wait) <= 1:
                return
            engine = getattr(inst, "engine", None)
            if engine is None or engine == mybir.EngineType.Unassigned:
                return
            waits = list(si.on_wait)
            for w in waits[:-1]:
                nop = mybir.InstNoOp(
                    name=self.nc.get_next_instruction_name(),
                    sync_info=mybir.SyncInfo(on_wait=[w], on_update=[]),
                    bass_nofuse=True,
                    engine=engine,
                )
                super()._commit_instruction(nop, lazy_reg_writes=False)
            inst.sync_info = mybir.SyncInfo(
                on_wait=[waits[-1]], on_update=list(si.on_update or [])
            )

        def _commit_instruction(self, inst, lazy_reg_writes: bool = True):
            if isinstance(inst, mybir.Instruction):
                self._split_waits(inst)
            super()._commit_instruction(inst, lazy_reg_writes)

        def _drain_and_barrier(self, tick_clock, wait_clock):
            nc = self.nc
            drain_inst = nc.sync.drain()
            wait_clock.add_sem_waits(
                drain_inst.ins, ScopedClock({None: tick_clock.global_clock})
            )
            si = drain_inst.ins.sync_info
            if si is not None and si.on_wait and len(si.on_wait) > 1:
                waits = list(si.on_wait)
                drain_inst.ins.sync_info = mybir.SyncInfo(
                    on_wait=waits[:1], on_update=list(si.on_update or [])
                )
                # spread the remaining frontier waits across engines so they
                # retire in parallel instead of serializing on SP
                engines = [nc.sync, nc.tensor, nc.vector, nc.scalar, nc.gpsimd]
                for i, w in enumerate(waits[1:]):
                    d2 = engines[i % len(engines)].drain()
                    d2.ins.sync_info = mybir.SyncInfo(on_wait=[w], on_update=[])
            nc.all_engine_barrier()
            assert self.sems is not None
            popped = nc._tile_sem_poison_stack.pop()
            assert popped is self._sem_poison
            # No clear_and_free_semaphores + trailing barrier: this is the
            # only TileContext in the program and the NEFF epilogue resets
            # all declared semaphores anyway; skipping shaves ~1.5us of
            # teardown inside the measured window.

    return OneWaitTC(nc)


def token_tiles(C: int):
    if C <= 0:
        return []
    tiles = [TOK] * (C // TOK)
    if C % TOK:
        tiles.append(C % TOK)
    return tiles


def build_moe_expert_kernel(cfg):
    """One SPMD program: SwiGLU FFN over B primary tokens (weight set 1)
    plus an R-token overflow tile (weight set 2)."""
    import concourse.bass as bass
    import concourse.mybir as mybir

    B, R = cfg if isinstance(cfg, tuple) else (cfg, 0)
    dt = mybir.dt
    nc = bass.Bass()
    C = B + R

    # prepacked layouts (see pack_* helpers below); xt is packed per token
    # tile ([P, KD*tok] blocks) so each tile's DMA is one contiguous
    # 8KB-per-partition read instead of 8 strided 1KB lines
    xt = nc.dram_tensor("xt", [P, B * KD], dt.bfloat16, kind="ExternalInput")
    wg = nc.dram_tensor("wg", [P, KH, KD, P], dt.bfloat16, kind="ExternalInput")
    wu = nc.dram_tensor("wu", [P, KH, KD, P], dt.bfloat16, kind="ExternalInput")
    wd = nc.dram_tensor("wd", [P, KD, KH, P], dt.bfloat16, kind="ExternalInput")
    if R:
        xt2 = nc.dram_tensor("xt2", [P, R * KD], dt.bfloat16, kind="ExternalInput")
        wg2 = nc.dram_tensor("wg2", [P, KH, KD, P], dt.bfloat16, kind="ExternalInput")
        wu2 = nc.dram_tensor("wu2", [P, KH, KD, P], dt.bfloat16, kind="ExternalInput")
        wd2 = nc.dram_tensor("wd2", [P, KD, KH, P], dt.bfloat16, kind="ExternalInput")
    yt = nc.dram_tensor("yt", [P, KD, C], dt.bfloat16, kind="ExternalOutput")

    tiles = token_tiles(B)
    WARMUP_MM = 50  # un-throttles HAM and covers the DMA ramp as ONE open
    # accumulation chain: start/stop-per-matmul warmups serialize on PE->PE
    # WAW semaphore round-trips (~560ns gaps); an open chain runs gap-free
    # at 216ns/matmul, so it ramps HAM faster and bridges solid to the
    # ~15us point where the first gate chain's data+sems are ready (fewer
    # dummies idle the PE and HAM demotes to half clock)

    with _make_tile_context(nc) as tc:
        with (
            tc.tile_pool(name="weights", bufs=1) as wpool,
            tc.tile_pool(name="xin0", bufs=1) as xpool0,
            tc.tile_pool(name="xin", bufs=2) as xpool,
            tc.tile_pool(name="h2", bufs=2) as hpool,
            tc.tile_pool(name="sg", bufs=4) as spool,
            tc.tile_pool(name="sgk", bufs=1) as kpool,
            tc.tile_pool(name="out", bufs=4) as opool,
            tc.tile_pool(name="psA", bufs=3, space="PSUM") as psA,
            tc.tile_pool(name="psB", bufs=2, space="PSUM") as psB,
        ):
            # --- PE warmup: dummy matmuls on a memset tile so the HAM
            # clock-gate reaches 8/8 (2.4 GHz) and the cold-clock penalty is
            # paid on throwaway work while the first weight/x DMAs land.
            warm_sb = xpool0.tile([P, TOK], dt.bfloat16, tag="warm", name="warm")
            nc.gpsimd.memset(warm_sb[:], 0.0)
            warm_ps = psB.tile([P, TOK], dt.float32, tag="py", name="wps")
            for i in range(WARMUP_MM):
                nc.tensor.matmul(
                    warm_ps[:], warm_sb[:, :P], warm_sb[:],
                    start=(i == 0), stop=(i == WARMUP_MM - 1),
                )

            # one tile per 128-col weight block: tiles are Tile's dependency
            # unit, so the m=0 matmuls only wait for their own block. The m=0
            # gate/up blocks are further split in half (separate tiles) so the
            # very first matmuls wait on a 128KB transfer, not 256KB.
            wg0a = wpool.tile([P, KD // 2, P], dt.bfloat16, tag="wg0a", name="wg0a")
            wg0b = wpool.tile([P, KD // 2, P], dt.bfloat16, tag="wg0b", name="wg0b")
            wu0 = wpool.tile([P, KD, P], dt.bfloat16, tag="wu0", name="wu0")
            wg_sb = [
                wpool.tile([P, KD, P], dt.bfloat16, tag=f"wg{m}", name=f"wg{m}")
                for m in range(1, KH)
            ]
            wu_sb = [
                wpool.tile([P, KD, P], dt.bfloat16, tag=f"wu{m}", name=f"wu{m}")
                for m in range(1, KH)
            ]
            wd_sb = [
                wpool.tile([P, KH, P], dt.bfloat16, tag=f"wd{m2}", name=f"wd{m2}")
                for m2 in range(KD)
            ]
            # overflow weight set: reuses the primary weight buffers (same
            # pool tags); each block's DMA waits on that block's last
            # primary read, ~60-80us before the overflow tile needs it.
            if R:
                wg2_0a = wpool.tile([P, KD // 2, P], dt.bfloat16, tag="wg0a",
                                    name="wg2_0a")
                wg2_0b = wpool.tile([P, KD // 2, P], dt.bfloat16, tag="wg0b",
                                    name="wg2_0b")
                wu2_0 = wpool.tile([P, KD, P], dt.bfloat16, tag="wu0", name="wu2_0")
                wg2_sb = [
                    wpool.tile([P, KD, P], dt.bfloat16, tag=f"wg{m}", name=f"wg2_{m}")
                    for m in range(1, KH)
                ]
                wu2_sb = [
                    wpool.tile([P, KD, P], dt.bfloat16, tag=f"wu{m}", name=f"wu2_{m}")
                    for m in range(1, KH)
                ]
                wd2_sb = [
                    wpool.tile([P, KH, P], dt.bfloat16,
                               tag=f"wd{m2}",
                               name=f"wd2_{m2}")
                    for m2 in range(KD)
                ]

            def wg_at(m, k):
                if m == 0:
                    return wg0a[:, k] if k < KD // 2 else wg0b[:, k - KD // 2]
                return wg_sb[m - 1][:, k]

            def wu_at(m, k):
                if m == 0:
                    return wu0[:, k]
                return wu_sb[m - 1][:, k]

            def wg2_at(m, k):
                if m == 0:
                    return wg2_0a[:, k] if k < KD // 2 else wg2_0b[:, k - KD // 2]
                return wg2_sb[m - 1][:, k]

            def wu2_at(m, k):
                if m == 0:
                    return wu2_0[:, k]
                return wu2_sb[m - 1][:, k]

            # --- DMA issue plan. All queues round-robin 2KB packets over the
            # shared DMA engines (~370 GB/s aggregate), so what matters is
            # (a) issue the critical-prefix transfers first and (b) split
            # them across several queues so per-transfer latency is low.
            #   scalar+vector HWDGE: tile-0 tokens in 4 quarter pieces
            #   sync HWDGE:          weights in consumption order
            #   scalar HWDGE tail:   overflow tokens + weight set 2
            xt_tiles = []
            xt023_pending = []
            off = 0
            for t_i, tok in enumerate(tiles):
                src = xt[:, off * KD : (off + tok) * KD].rearrange(
                    "p (k t) -> p k t", k=KD
                )
                if t_i == 0:
                    # 4 quarter-tiles: first two on the ACT HWDGE ring, last
                    # two on the GpSimd SWDGE ring (issued before the xt1..8
                    # loads), so the k<2 matmuls wait only on a 256KB
                    # transfer and the rest land while they run
                    q = KD // 4
                    pieces = []
                    for pi in range(2):
                        pt = xpool0.tile(
                            [P, q, TOK], dt.bfloat16, tag=f"xt0{pi}",
                            name=f"xt0{pi}",
                        )
                        nc.scalar.dma_start(pt[:, :, :tok], src[:, pi * q : (pi + 1) * q])
                        pieces.append(pt)
                    # back half split across both HWDGE rings (k4,k5 as a
                    # third scalar piece; k6,k7 on sync right after wg0) --
                    # zero SWDGE DMAs keeps the lane-semaphore count low and
                    # no single ramp gap crosses the 3.4us HAM window
                    pt2 = xpool0.tile(
                        [P, q, TOK], dt.bfloat16, tag="xt02", name="xt02"
                    )
                    nc.scalar.dma_start(pt2[:, :, :tok], src[:, 2 * q : 3 * q])
                    pieces.append(pt2)
                    pt3 = xpool0.tile(
                        [P, q, TOK], dt.bfloat16, tag="xt03", name="xt03"
                    )
                    xt023_pending.append((pt3, src))
                    pieces.append(pt3)
                    xt_tiles.append(pieces)
                elif t_i == 1:
                    # dedicated tag + issued on the sync ring after the
                    # gate/up weights: keeps this 1MB prefetch off the
                    # ramp-critical window without pool-slot cycles
                    xt_sb = xpool0.tile(
                        [P, KD, TOK], dt.bfloat16, tag="xt1", name="xt1"
                    )
                    xt1_pending = (xt_sb, src, tok)
                    xt_tiles.append(xt_sb)
                else:
                    xt_sb = xpool.tile(
                        [P, KD, TOK], dt.bfloat16, tag="xt", name=f"xt{off}"
                    )
                    nc.scalar.dma_start(xt_sb[:, :, :tok], src)
                    xt_tiles.append(xt_sb)
                off += tok

            # overflow tokens: tiny (R*KD*2 bytes/partition), own tag, no
            # waits; rides the scalar ring behind the primary token tiles.
            if R:
                xt2_sb = xpool0.tile([P, KD, R], dt.bfloat16, tag="xt2ov",
                                     name="xt2ov")
                nc.scalar.dma_start(
                    xt2_sb[:], xt2[:].rearrange("p (k t) -> p k t", k=KD)
                )

            # weights in exact first-use order. wu0 rides the ACT ring
            # (after the xt0 pieces) so the m=0 up-chain is not queued
            # behind wg1.. on the serialized sync ring.
            half = KD // 2
            nc.sync.dma_start(wg0a[:], wg[:, 0, :half])
            nc.sync.dma_start(wg0b[:], wg[:, 0, half:])
            q4 = KD // 4
            tok0 = tiles[0]
            for pt3, src0 in xt023_pending:
                nc.sync.dma_start(pt3[:, :, :tok0], src0[:, 3 * q4 :])
            for m in range(1, KH):
                nc.sync.dma_start(wg_sb[m - 1][:], wg[:, m])
            # up weights (needed from ~42us) follow the gate weights on the
            # sync ring: the ACT ring goes idle after the xt0 pieces, so the
            # full DMA fabric serves the gate stream during the ramp-stall
            # window (~15-20us) instead of prefetching wu early.
            nc.sync.dma_start(wu0[:], wu[:, 0])
            for m in range(1, KH):
                nc.sync.dma_start(wu_sb[m - 1][:], wu[:, m])
            for m2 in range(KD):
                nc.sync.dma_start(wd_sb[m2][:], wd[:, m2])
            xt1_sb, xt1_src, xt1_tok = xt1_pending
            nc.sync.dma_start(xt1_sb[:, :, :xt1_tok], xt1_src)
            # overflow tokens + weight set 2, emitted AFTER the primary
            # weight DMAs so the shared-tag write order is primary ->
            # overflow. The weight transfers WAIT on each buffer's last
            # primary read (~600us); a dma_start blocks its issuing
            # engine's stream until the wait clears, so they ride the
            # GpSimd SWDGE ring (idle after the warmup memset) -- scalar
            # still owes every silu and sync the yt output writes.
            if R:
                # per-block interleave matches the order the buffer-reuse
                # waits clear during tile-7's gate/up sweep, so the in-order
                # queue never head-of-line blocks a ready transfer
                nc.gpsimd.dma_start(wg2_0a[:], wg2[:, 0, :half])
                nc.gpsimd.dma_start(wg2_0b[:], wg2[:, 0, half:])
                nc.gpsimd.dma_start(wu2_0[:], wu2[:, 0])
                for m in range(1, KH):
                    nc.gpsimd.dma_start(wg2_sb[m - 1][:], wg2[:, m])
                    nc.gpsimd.dma_start(wu2_sb[m - 1][:], wu2[:, m])
                for m2 in range(KD):
                    nc.gpsimd.dma_start(wd2_sb[m2][:], wd2[:, m2])

            # Overflow work is interleaved into tile-7's dense stream so the
            # PE duty never drops low enough for HAM to demote the clock
            # (a separate sparse 44-col region measured 7-16us at half
            # clock). Offsets: ovf gate/up block m-5 after tile-7 block m
            # (weight-set-2 block m-5 transferred ~5 block-times earlier);
            # ovf blocks 11..15 and the ovf down chains ride the down sweep.
            if R:
                h2o = [
                    kpool.tile([P, TOK], dt.bfloat16, tag=f"sgk{m}",
                               name=f"h2o{m}")
                    for m in range(KH)
                ]

                def ovf_gate_up_block(m):
                    pg = psA.tile([P, TOK], dt.float32, tag="pg", name=f"pgo_{m}")
                    for k in range(KD):
                        nc.tensor.matmul(
                            pg[:, :R], wg2_at(m, k), xt2_sb[:, k],
                            start=(k == 0), stop=(k == KD - 1),
                        )
                    pu = psA.tile([P, TOK], dt.float32, tag="pu", name=f"puo_{m}")
                    for k in range(KD):
                        nc.tensor.matmul(
                            pu[:, :R], wu2_at(m, k), xt2_sb[:, k],
                            start=(k == 0), stop=(k == KD - 1),
                        )
                    pgs = spool.tile([P, TOK], dt.float32, tag="pgs")
                    nc.vector.tensor_copy(pgs[:, :R], pg[:, :R])
                    pus = spool.tile([P, TOK], dt.float32, tag="pus")
                    nc.vector.tensor_copy(pus[:, :R], pu[:, :R])
                    sgo = spool.tile([P, TOK], dt.bfloat16, tag="sg")
                    nc.scalar.activation(
                        sgo[:, :R], pgs[:, :R],
                        mybir.ActivationFunctionType.Silu,
                    )
                    nc.vector.tensor_mul(h2o[m][:, :R], sgo[:, :R], pus[:, :R])

                def ovf_down_block(m2):
                    py = psB.tile([P, TOK], dt.float32, tag="py", name=f"pyo_{m2}")
                    for k2 in range(KH):
                        nc.tensor.matmul(
                            py[:, :R], wd2_sb[m2][:, k2], h2o[k2][:, :R],
                            start=(k2 == 0), stop=(k2 == KH - 1),
                        )
                    ot = opool.tile([P, TOK], dt.bfloat16, tag="ot")
                    nc.vector.tensor_copy(ot[:, :R], py[:, :R])
                    nc.sync.dma_start(yt[:, m2, B : B + R], ot[:, :R])

            last_t = len(tiles) - 1
            # ovf emission points live in the generic (t_i > 0) branch
            assert not R or last_t >= 1, "overflow needs >= 2 primary tiles"
            off = 0
            for t_i, tok in enumerate(tiles):
                ts = slice(off, off + tok)
                off += tok
                xt_sb = xt_tiles[t_i]
                if t_i == 0:
                    q = KD // 4
                    rhs = lambda k, _p=xt_sb: _p[k // q][:, k % q]
                else:
                    rhs = lambda k, _x=xt_sb: _x[:, k]

                # h2 is one tile for full tiles: the first down matmul then
                # carries a single wait (no hoisted-NoOp slot on the PE
                # queue, which costs a full 216ns each). Only a final ragged
                # tile (R == 0 case) uses per-m tiles, letting its down
                # chains start before the last silu/mul retires (tail trim).
                last = t_i == len(tiles) - 1 and tok < TOK and not R
                if last:
                    h2_sb = [
                        kpool.tile([P, TOK], dt.bfloat16, tag=f"sgk{m}", name=f"h2m{m}")
                        for m in range(KH)
                    ]
                    h2w = lambda m: h2_sb[m][:, :tok]
                    h2r = lambda k2: h2_sb[k2][:, :tok]
                else:
                    h2_sb = hpool.tile([P, KH, TOK], dt.bfloat16, tag="h2",
                                       name=f"h2_{off}")
                    h2w = lambda m: h2_sb[:, m, :tok]
                    h2r = lambda k2: h2_sb[:, k2, :tok]
                if t_i == 0:
                    # first tile: ALL gate chains, then ALL up chains, so the
                    # m=0 up weights are not needed until ~42us into the
                    # kernel - matching what the DMA rings can deliver. The
                    # silu output for each m is kept in a dedicated tile
                    # until the up sweep reaches it.
                    sg_keep = []
                    for m in range(KH):
                        pg = psA.tile([P, TOK], dt.float32, tag="pg", name=f"pg0_{m}")
                        for k in range(KD):
                            nc.tensor.matmul(
                                pg[:, :tok], wg_at(m, k), rhs(k)[:, :tok],
                                start=(k == 0), stop=(k == KD - 1),
                            )
                        pgs = spool.tile([P, TOK], dt.float32, tag="pgs")
                        nc.vector.tensor_copy(pgs[:, :tok], pg[:, :tok])
                        sgm = kpool.tile(
                            [P, TOK], dt.bfloat16, tag=f"sgk{m}", name=f"sgk{m}"
                        )
                        nc.scalar.activation(
                            sgm[:, :tok], pgs[:, :tok],
                            mybir.ActivationFunctionType.Silu,
                        )
                        sg_keep.append(sgm)
                    for m in range(KH):
                        pu = psA.tile([P, TOK], dt.float32, tag="pu", name=f"pu0_{m}")
                        for k in range(KD):
                            nc.tensor.matmul(
                                pu[:, :tok], wu_at(m, k), rhs(k)[:, :tok],
                                start=(k == 0), stop=(k == KD - 1),
                            )
                        pus = spool.tile([P, TOK], dt.float32, tag="pus")
                        nc.vector.tensor_copy(pus[:, :tok], pu[:, :tok])
                        nc.vector.tensor_mul(
                            h2w(m), sg_keep[m][:, :tok], pus[:, :tok]
                        )
                else:
                    for m in range(KH):
                        pg = psA.tile([P, TOK], dt.float32, tag="pg", name=f"pg{off}_{m}")
                        for k in range(KD):
                            nc.tensor.matmul(
                                pg[:, :tok], wg_at(m, k), rhs(k)[:, :tok],
                                start=(k == 0), stop=(k == KD - 1),
                            )
                        pu = psA.tile([P, TOK], dt.float32, tag="pu", name=f"pu{off}_{m}")
                        for k in range(KD):
                            nc.tensor.matmul(
                                pu[:, :tok], wu_at(m, k), rhs(k)[:, :tok],
                                start=(k == 0), stop=(k == KD - 1),
                            )
                        # fast DVE copies release the PSUM banks immediately;
                        # silu+mul then run off SBUF, off the bank-recycle path
                        pgs = spool.tile([P, TOK], dt.float32, tag="pgs")
                        nc.vector.tensor_copy(pgs[:, :tok], pg[:, :tok])
                        pus = spool.tile([P, TOK], dt.float32, tag="pus")
                        nc.vector.tensor_copy(pus[:, :tok], pu[:, :tok])
                        sg = spool.tile([P, TOK], dt.bfloat16, tag="sg")
                        nc.scalar.activation(
                            sg[:, :tok], pgs[:, :tok],
                            mybir.ActivationFunctionType.Silu,
                        )
                        nc.vector.tensor_mul(
                            h2w(m), sg[:, :tok], pus[:, :tok]
                        )
                        if R and t_i == last_t and m >= 5:
                            ovf_gate_up_block(m - 5)

                for m2 in range(KD):
                    py = psB.tile([P, TOK], dt.float32, tag="py", name=f"py{off}_{m2}")
                    for k2 in range(KH):
                        nc.tensor.matmul(
                            py[:, :tok], wd_sb[m2][:, k2], h2r(k2),
                            start=(k2 == 0), stop=(k2 == KH - 1),
                        )
                    ot = opool.tile([P, TOK], dt.bfloat16, tag="ot")
                    nc.vector.tensor_copy(ot[:, :tok], py[:, :tok])
                    nc.sync.dma_start(yt[:, m2, ts], ot[:, :tok])
                    if R and t_i == last_t:
                        if m2 <= 4:
                            ovf_gate_up_block(11 + m2)
                        else:
                            ovf_down_block(m2 - 5)
                if R and t_i == last_t:
                    for m2o in range(3, KD):
                        ovf_down_block(m2o)

    return nc


def pack_lhsT(w: np.ndarray) -> np.ndarray:
    """[K, M] weight -> [p=128, m_block, k_chunk, 128] bf16, so that
    slice [:, m, k, :] is the lhsT tile for contraction chunk k, output
    block m, and each [:, m] block is one contiguous DMA."""
    K, M = w.shape
    kc, mb = K // P, M // P
    return np.ascontiguousarray(
        w.reshape(kc, P, mb, P).transpose(1, 2, 0, 3)
    ).astype(_BF16)


def pack_tokens(xe: np.ndarray, C: int) -> np.ndarray:
    """[n, D] tokens -> zero-padded [p=128, C*KD] bf16, blocked per token
    tile as [KD, tok] per partition (one contiguous DMA per tile)."""
    n = xe.shape[0]
    out = np.zeros((P, C * KD), dtype=_BF16)
    off = 0
    for tok in token_tiles(C):
        xe_t = xe[off : min(off + tok, n)]
        nt = xe_t.shape[0]
        if nt:
            blk = np.zeros((P, KD, tok), dtype=_BF16)
            # [nt, D] -> [D, nt] -> [KD, P, nt] -> [P, KD, nt]
            blk[:, :, :nt] = (
                xe_t.T.reshape(KD, P, nt).transpose(1, 0, 2).astype(_BF16)
            )
            out[:, off * KD : (off + tok) * KD] = blk.reshape(P, KD * tok)
        off += tok
    return out


def route_tokens(xf: np.ndarray, router_w: np.ndarray):
    """Top-2 routing identical to the reference (softmax over selected)."""
    logits = xf @ router_w  # [T, E]
    # top-2 per token (order irrelevant: softmax over the pair + scatter)
    top_idx = np.argpartition(-logits, TOP_K, axis=-1)[:, :TOP_K]
    tv = np.take_along_axis(logits, top_idx, axis=-1)
    tv = tv - tv.max(axis=-1, keepdims=True)
    ev = np.exp(tv)
    probs = ev / ev.sum(axis=-1, keepdims=True)

    idx, scale = [], []
    for e in range(NUM_EXPERTS):
        hit = top_idx == e  # [T, 2]
        rows = np.nonzero(hit.any(axis=-1))[0]
        w = np.where(hit[rows, 0], probs[rows, 0], probs[rows, 1])
        idx.append(rows)
        scale.append(w.astype(np.float32))
    return idx, scale


def plan_overflow(idx):
    """Split each expert's tokens beyond B_PRIMARY into chunks of at most
    R (minimal R with at most NUM_EXPERTS chunks) and assign one chunk
    per core. Returns (B, R, chunks) with chunks[c] = (expert, lo, hi)
    slicing idx[expert][lo:hi], or None."""
    n_e = [len(r) for r in idx]
    if max(n_e) <= B_PRIMARY:
        return min(B_PRIMARY, max(n_e)), 0, [None] * NUM_EXPERTS
    B = B_PRIMARY
    o = [max(n - B, 0) for n in n_e]
    R = None
    for r in range(1, max(o) + 1):
        if sum(-(-oe // r) for oe in o if oe) <= NUM_EXPERTS:
            R = r
            break
    chunks = []
    for e in range(NUM_EXPERTS):
        lo = B
        while lo < n_e[e]:
            hi = min(lo + R, n_e[e])
            chunks.append((e, lo, hi))
            lo = hi
    assert len(chunks) <= NUM_EXPERTS
    chunks += [None] * (NUM_EXPERTS - len(chunks))
    return B, R, chunks


def prepare_in_maps(x, router_w, w_gate, w_up, w_down):
    x = np.asarray(x, dtype=np.float32)
    xf = x.reshape(-1, EMB)
    idx, scale = route_tokens(xf, np.asarray(router_w, dtype=np.float32))
    B, R, chunks = plan_overflow(idx)

    wg_all = np.asarray(w_gate, dtype=np.float32)
    wu_all = np.asarray(w_up, dtype=np.float32)
    wd_all = np.asarray(w_down, dtype=np.float32)
    packed = [
        (pack_lhsT(wg_all[e]), pack_lhsT(wu_all[e]), pack_lhsT(wd_all[e]))
        for e in range(NUM_EXPERTS)
    ]

    in_maps = []
    for c in range(NUM_EXPERTS):
        im = {
            "xt": pack_tokens(xf[idx[c][:B]], B),
            "wg": packed[c][0],
            "wu": packed[c][1],
            "wd": packed[c][2],
        }
        if R:
            ch = chunks[c]
            e2 = ch[0] if ch else c
            rows = idx[e2][ch[1]:ch[2]] if ch else np.zeros(0, dtype=np.int64)
            im["xt2"] = pack_tokens(xf[rows], R)
            im["wg2"] = packed[e2][0]
            im["wu2"] = packed[e2][1]
            im["wd2"] = packed[e2][2]
        in_maps.append(im)
    return in_maps, idx, scale, (B, R), xf


def kernel(x, router_w, w_gate, w_up, w_down):
    from concourse.bass_utils import run_bass_kernel_spmd

    in_maps, idx, scale, cfg, xf = prepare_in_maps(
        x, router_w, w_gate, w_up, w_down
    )
    B, R = cfg
    _, _, chunks = plan_overflow(idx)
    nc = build_moe_expert_kernel(cfg)
    res = None
    last_exc = None
    for _attempt in range(3):
        try:
            res = run_bass_kernel_spmd(nc, in_maps, list(range(NUM_EXPERTS)))
            break
        except Exception as exc:  # transient device wedge: retry
            last_exc = exc
    if res is None:
        raise last_exc

    C = B + R
    out = np.zeros_like(xf)
    for c in range(NUM_EXPERTS):
        ytc = np.asarray(res.results[c]["yt"]).astype(np.float32)  # [P, KD, C]
        y = ytc.transpose(1, 0, 2).reshape(EMB, C)  # [D, C]
        nb = min(len(idx[c]), B)
        # indices within one scatter are unique -> fancy += is safe;
        # primary and overflow scatters are done separately because a
        # token may appear in both (different experts).
        out[idx[c][:nb]] += y[:, :nb].T * scale[c][:nb, None]
        if R and chunks[c]:
            e2, lo, hi = chunks[c]
            out[idx[e2][lo:hi]] += y[:, B : B + hi - lo].T * scale[e2][lo:hi, None]
    return out.reshape(np.asarray(x).shape)


# revision 30
# speedup vs baseline: 1.0068x; 1.0068x over previous
"""MoE (top-2 of 8 experts, SwiGLU) Trainium2 kernel.

Strategy (expert parallelism + overflow-tile load balancing):
  - Host: compute router logits/top-2/softmax, dispatch tokens to
    experts. Every core gets B=4096 primary tokens of its own expert
    (the global mean load is exactly 4096 = 2*16384/8). Each expert's
    tokens beyond B ("overflow", ~256 total here) are chopped into
    chunks of at most R (minimal R with sum_e ceil(o_e/R) <= 8, R=44
    for this routing) and each chunk is shipped to some core's single
    overflow tile together with THAT expert's weight set. This removes
    the load imbalance (max_e n_e = 4213 would otherwise pad every
    core) at the cost of one extra ragged tile and a second streamed
    weight set per core.
  - Device: 8 NeuronCores; core c runs its expert's SwiGLU FFN over 8
    full 512-token tiles, then the R-token overflow tile with the
    second weight set. All matmuls bf16 with fp32 PSUM accumulation.
  - Host: weighted scatter-add combine (two disjoint scatters: primary
    and overflow; each has unique row indices).

Device compute per core (transposed so every matmul uses natural,
transpose-free operand layouts; PSUM accumulates over the contraction):
  hT[h_chunk, tok] = wg.T @ xt   (accumulate K=D over 8 chunks of 128)
  h2 = silu(hT_gate) * hT_up     (ACT silu + DVE mul, bf16 out)
  yT[d_chunk, tok] = wd.T @ h2   (accumulate K=H over 16 chunks of 128)

The steady state runs at the PE roofline cadence (512 cols / 2.4 GHz +
2.5ns NX issue = 216ns per matmul, measured); the overheads around it
are attacked as follows:
  - PE warmup: 14 dummy matmuls on a memset tile run during the DMA ramp
    so the HAM clock-gate reaches 8/8 (2.4 GHz) before real work and the
    cold 1.2 GHz cycles are spent on throwaway work.
  - Tile 0 computes all 16 GATE chains, then all 16 UP chains: the
    up-weight DMA deadline moves from ~16us (undeliverable while the DMA
    rings spin up) to ~42us. Weights stream in exact consumption order.
  - Ring allocation: the ACT ring carries ONLY the xt0 k0..k5 pieces and
    goes idle by ~13us; everything else (wg0 halves, xt0 k6..7, wg1..15,
    wu0..15, wd0..7, xt1) streams on the sync ring in consumption order,
    so the full DMA fabric serves the gate stream during the ramp-stall
    window. Later token tiles prefetch behind it.
  - The overflow weight set streams into the SAME SBUF buffers (pool
    tags) as the primary weights, with each block's DMA gated on that
    block's last primary read; transfers ride the otherwise-idle scalar
    ring from ~600us and land well before the overflow tile needs them.
  - Zero SWDGE transfers: 8 fewer DMA-lane semaphores shrink the
    per-engine NEFF epilogue inside the measured window, and the exit
    skips the redundant semaphore-clear + second barrier.
  - bf16 output (error budget allows it: rel_err 4.7e-3 vs 2e-2 gate).

Weights are host-prepacked to [p=128, out_block, k_chunk, 128] so each
128-column weight block is one contiguous DMA.

NOTE on fp8: DoubleRow fp8 matmuls were prototyped (2x PE rate, error
budget fits a low-routing-weight token subset) but the mere presence of
perf-mode matmuls in a program serializes LDWEIGHTS against MATMUL for
every bf16 instruction (+43ns each, measured 216->259ns), costing more
than the fp8 region saves at any error-feasible split. See
kernel_fp8_attempt.py.
"""

import sys

if "/opt/trn_rl_repo" not in sys.path:
    sys.path.insert(0, "/opt/trn_rl_repo")

import ml_dtypes
import numpy as np

NUM_EXPERTS = 8
TOP_K = 2
EMB = 1024
HID = 2048
P = 128
KD = EMB // P  # 8
KH = HID // P  # 16
TOK = 512  # main token tile (one PSUM bank of f32)
B_PRIMARY = 4096  # per-core primary capacity == mean load (2*T/8)

_BF16 = ml_dtypes.bfloat16


def _make_tile_context(nc):
    """TileContext whose emitted instructions carry at most ONE sem wait.

    The walrus codegen bundled in this environment rejects any instruction
    with more than one sync-wait command ("Too many sync wait commands").
    Tile's scheduler freely attaches several waits to one instruction (and
    its exit drain waits on every frontier semaphore), so hoist all but the
    last wait onto dedicated same-engine NoOps immediately preceding the
    instruction.
    """
    import concourse.mybir as mybir
    import concourse.tile as tile
    from concourse.vector_clock import ScopedClock

    class OneWaitTC(tile.TileContext):
        def _split_waits(self, inst):
            si = getattr(inst, "sync_info", None)
            if si is None or not si.on_wait or len(si.on_wait) <= 1:
                return
            engine = getattr(inst, "engine", None)
            if engine is None or engine == mybir.EngineType.Unassigned:
                return
            waits = list(si.on_wait)
            for w in waits[:-1]:
                nop = mybir.InstNoOp(
                    name=self.nc.get_next_instruction_name(),
                    sync_info=mybir.SyncInfo(on_wait=[w], on_update=[]),
                    bass_nofuse=True,
                    engine=engine,
                )
                super()._commit_instruction(nop, lazy_reg_writes=False)
            inst.sync_info = mybir.SyncInfo(
                on_wait=[waits[-1]], on_update=list(si.on_update or [])
            )

        def _commit_instruction(self, inst, lazy_reg_writes: bool = True):
            if isinstance(inst, mybir.Instruction):
                self._split_waits(inst)
            super()._commit_instruction(inst, lazy_reg_writes)

        def _drain_and_barrier(self, tick_clock, wait_clock):
            nc = self.nc
            drain_inst = nc.sync.drain()
            wait_clock.add_sem_waits(
                drain_inst.ins, ScopedClock({None: tick_clock.global_clock})
            )
            si = drain_inst.ins.sync_info
            if si is not None and si.on_wait and len(si.on_wait) > 1:
                waits = list(si.on_wait)
                drain_inst.ins.sync_info = mybir.SyncInfo(
                    on_wait=waits[:1], on_update=list(si.on_update or [])
                )
                # spread the remaining frontier waits across engines so they
                # retire in parallel instead of serializing on SP
                engines = [nc.sync, nc.tensor, nc.vector, nc.scalar, nc.gpsimd]
                for i, w in enumerate(waits[1:]):
                    d2 = engines[i % len(engines)].drain()
                    d2.ins.sync_info = mybir.SyncInfo(on_wait=[w], on_update=[])
            nc.all_engine_barrier()
            assert self.sems is not None
            popped = nc._tile_sem_poison_stack.pop()
            assert popped is self._sem_poison
            # No clear_and_free_semaphores + trailing barrier: this is the
            # only TileContext in the program and the NEFF epilogue resets
            # all declared semaphores anyway; skipping shaves ~1.5us of
            # teardown inside the measured window.

    return OneWaitTC(nc)


def token_tiles(C: int):
    if C <= 0:
        return []
    tiles = [TOK] * (C // TOK)
    if C % TOK:
        tiles.append(C % TOK)
    return tiles


def build_moe_expert_kernel(cfg):
    """One SPMD program: SwiGLU FFN over B primary tokens (weight set 1)
    plus an R-token overflow tile (weight set 2)."""
    import concourse.bass as bass
    import concourse.mybir as mybir

    B, R = cfg if isinstance(cfg, tuple) else (cfg, 0)
    dt = mybir.dt
    nc = bass.Bass()
    C = B + R

    # prepacked layouts (see pack_* helpers below); xt is packed per token
    # tile ([P, KD*tok] blocks) so each tile's DMA is one contiguous
    # 8KB-per-partition read instead of 8 strided 1KB lines
    xt = nc.dram_tensor("xt", [P, B * KD], dt.bfloat16, kind="ExternalInput")
    wg = nc.dram_tensor("wg", [P, KH, KD, P], dt.bfloat16, kind="ExternalInput")
    wu = nc.dram_tensor("wu", [P, KH, KD, P], dt.bfloat16, kind="ExternalInput")
    wd = nc.dram_tensor("wd", [P, KD, KH, P], dt.bfloat16, kind="ExternalInput")
    if R:
        xt2 = nc.dram_tensor("xt2", [P, R * KD], dt.bfloat16, kind="ExternalInput")
        wg2 = nc.dram_tensor("wg2", [P, KH, KD, P], dt.bfloat16, kind="ExternalInput")
        wu2 = nc.dram_tensor("wu2", [P, KH, KD, P], dt.bfloat16, kind="ExternalInput")
        wd2 = nc.dram_tensor("wd2", [P, KD, KH, P], dt.bfloat16, kind="ExternalInput")
    yt = nc.dram_tensor("yt", [P, KD, C], dt.bfloat16, kind="ExternalOutput")

    tiles = token_tiles(B)
    WARMUP_MM = 14  # un-throttles HAM and covers the DMA ramp: data lands
    # ~10.5us but sem-latency gates the first chain to ~15us; fewer dummies
    # (9, 12 tried) idle the PE and HAM demotes to half clock; an open-chain
    # warmup (50) and overflow-interleave variants both measured slower

    with _make_tile_context(nc) as tc:
        with (
            tc.tile_pool(name="weights", bufs=1) as wpool,
            tc.tile_pool(name="xin0", bufs=1) as xpool0,
            tc.tile_pool(name="xin", bufs=2) as xpool,
            tc.tile_pool(name="h2", bufs=2) as hpool,
            tc.tile_pool(name="sg", bufs=4) as spool,
            tc.tile_pool(name="sgk", bufs=1) as kpool,
            tc.tile_pool(name="out", bufs=4) as opool,
            tc.tile_pool(name="psA", bufs=3, space="PSUM") as psA,
            tc.tile_pool(name="psB", bufs=2, space="PSUM") as psB,
        ):
            # --- PE warmup: dummy matmuls on a memset tile so the HAM
            # clock-gate reaches 8/8 (2.4 GHz) and the cold-clock penalty is
            # paid on throwaway work while the first weight/x DMAs land.
            warm_sb = xpool0.tile([P, TOK], dt.bfloat16, tag="warm", name="warm")
            nc.gpsimd.memset(warm_sb[:], 0.0)
            warm_ps = psB.tile([P, TOK], dt.float32, tag="py", name="wps")
            for i in range(WARMUP_MM):
                nc.tensor.matmul(
                    warm_ps[:], warm_sb[:, :P], warm_sb[:], start=True, stop=True
                )

            # one tile per 128-col weight block: tiles are Tile's dependency
            # unit, so the m=0 matmuls only wait for their own block. The m=0
            # gate/up blocks are further split in half (separate tiles) so the
            # very first matmuls wait on a 128KB transfer, not 256KB.
            wg0a = wpool.tile([P, KD // 2, P], dt.bfloat16, tag="wg0a", name="wg0a")
            wg0b = wpool.tile([P, KD // 2, P], dt.bfloat16, tag="wg0b", name="wg0b")
            wu0 = wpool.tile([P, KD, P], dt.bfloat16, tag="wu0", name="wu0")
            wg_sb = [
                wpool.tile([P, KD, P], dt.bfloat16, tag=f"wg{m}", name=f"wg{m}")
                for m in range(1, KH)
            ]
            wu_sb = [
                wpool.tile([P, KD, P], dt.bfloat16, tag=f"wu{m}", name=f"wu{m}")
                for m in range(1, KH)
            ]
            wd_sb = [
                wpool.tile([P, KH, P], dt.bfloat16, tag=f"wd{m2}", name=f"wd{m2}")
                for m2 in range(KD)
            ]
            # overflow weight set: reuses the primary weight buffers (same
            # pool tags); each block's DMA waits on that block's last
            # primary read, ~60-80us before the overflow tile needs it.
            if R:
                wg2_0a = wpool.tile([P, KD // 2, P], dt.bfloat16, tag="wg0a",
                                    name="wg2_0a")
                wg2_0b = wpool.tile([P, KD // 2, P], dt.bfloat16, tag="wg0b",
                                    name="wg2_0b")
                wu2_0 = wpool.tile([P, KD, P], dt.bfloat16, tag="wu0", name="wu2_0")
                wg2_sb = [
                    wpool.tile([P, KD, P], dt.bfloat16, tag=f"wg{m}", name=f"wg2_{m}")
                    for m in range(1, KH)
                ]
                wu2_sb = [
                    wpool.tile([P, KD, P], dt.bfloat16, tag=f"wu{m}", name=f"wu2_{m}")
                    for m in range(1, KH)
                ]
                wd2_sb = [
                    wpool.tile([P, KH, P], dt.bfloat16,
                               tag=f"wd{m2}",
                               name=f"wd2_{m2}")
                    for m2 in range(KD)
                ]

            def wg_at(m, k):
                if m == 0:
                    return wg0a[:, k] if k < KD // 2 else wg0b[:, k - KD // 2]
                return wg_sb[m - 1][:, k]

            def wu_at(m, k):
                if m == 0:
                    return wu0[:, k]
                return wu_sb[m - 1][:, k]

            def wg2_at(m, k):
                if m == 0:
                    return wg2_0a[:, k] if k < KD // 2 else wg2_0b[:, k - KD // 2]
                return wg2_sb[m - 1][:, k]

            def wu2_at(m, k):
                if m == 0:
                    return wu2_0[:, k]
                return wu2_sb[m - 1][:, k]

            # --- DMA issue plan. All queues round-robin 2KB packets over the
            # shared DMA engines (~370 GB/s aggregate), so what matters is
            # (a) issue the critical-prefix transfers first and (b) split
            # them across several queues so per-transfer latency is low.
            #   scalar+vector HWDGE: tile-0 tokens in 4 quarter pieces
            #   sync HWDGE:          weights in consumption order
            #   scalar HWDGE tail:   overflow tokens + weight set 2
            xt_tiles = []
            xt023_pending = []
            off = 0
            for t_i, tok in enumerate(tiles):
                src = xt[:, off * KD : (off + tok) * KD].rearrange(
                    "p (k t) -> p k t", k=KD
                )
                if t_i == 0:
                    # 4 quarter-tiles: first two on the ACT HWDGE ring, last
                    # two on the GpSimd SWDGE ring (issued before the xt1..8
                    # loads), so the k<2 matmuls wait only on a 256KB
                    # transfer and the rest land while they run
                    q = KD // 4
                    pieces = []
                    for pi in range(2):
                        pt = xpool0.tile(
                            [P, q, TOK], dt.bfloat16, tag=f"xt0{pi}",
                            name=f"xt0{pi}",
                        )
                        nc.scalar.dma_start(pt[:, :, :tok], src[:, pi * q : (pi + 1) * q])
                        pieces.append(pt)
                    # back half split across both HWDGE rings (k4,k5 as a
                    # third scalar piece; k6,k7 on sync right after wg0) --
                    # zero SWDGE DMAs keeps the lane-semaphore count low and
                    # no single ramp gap crosses the 3.4us HAM window
                    pt2 = xpool0.tile(
                        [P, q, TOK], dt.bfloat16, tag="xt02", name="xt02"
                    )
                    nc.scalar.dma_start(pt2[:, :, :tok], src[:, 2 * q : 3 * q])
                    pieces.append(pt2)
                    pt3 = xpool0.tile(
                        [P, q, TOK], dt.bfloat16, tag="xt03", name="xt03"
                    )
                    xt023_pending.append((pt3, src))
                    pieces.append(pt3)
                    xt_tiles.append(pieces)
                elif t_i == 1:
                    # dedicated tag + issued on the sync ring after the
                    # gate/up weights: keeps this 1MB prefetch off the
                    # ramp-critical window without pool-slot cycles
                    xt_sb = xpool0.tile(
                        [P, KD, TOK], dt.bfloat16, tag="xt1", name="xt1"
                    )
                    xt1_pending = (xt_sb, src, tok)
                    xt_tiles.append(xt_sb)
                else:
                    xt_sb = xpool.tile(
                        [P, KD, TOK], dt.bfloat16, tag="xt", name=f"xt{off}"
                    )
                    nc.scalar.dma_start(xt_sb[:, :, :tok], src)
                    xt_tiles.append(xt_sb)
                off += tok

            # overflow tokens: tiny (R*KD*2 bytes/partition), own tag, no
            # waits; rides the scalar ring behind the primary token tiles.
            if R:
                xt2_sb = xpool0.tile([P, KD, R], dt.bfloat16, tag="xt2ov",
                                     name="xt2ov")
                nc.scalar.dma_start(
                    xt2_sb[:], xt2[:].rearrange("p (k t) -> p k t", k=KD)
                )

            # weights in exact first-use order. wu0 rides the ACT ring
            # (after the xt0 pieces) so the m=0 up-chain is not queued
            # behind wg1.. on the serialized sync ring.
            half = KD // 2
            nc.sync.dma_start(wg0a[:], wg[:, 0, :half])
            nc.sync.dma_start(wg0b[:], wg[:, 0, half:])
            q4 = KD // 4
            tok0 = tiles[0]
            for pt3, src0 in xt023_pending:
                nc.sync.dma_start(pt3[:, :, :tok0], src0[:, 3 * q4 :])
            for m in range(1, KH):
                nc.sync.dma_start(wg_sb[m - 1][:], wg[:, m])
            # up weights (needed from ~42us) follow the gate weights on the
            # sync ring: the ACT ring goes idle after the xt0 pieces, so the
            # full DMA fabric serves the gate stream during the ramp-stall
            # window (~15-20us) instead of prefetching wu early.
            nc.sync.dma_start(wu0[:], wu[:, 0])
            for m in range(1, KH):
                nc.sync.dma_start(wu_sb[m - 1][:], wu[:, m])
            for m2 in range(KD):
                nc.sync.dma_start(wd_sb[m2][:], wd[:, m2])
            xt1_sb, xt1_src, xt1_tok = xt1_pending
            nc.sync.dma_start(xt1_sb[:, :, :xt1_tok], xt1_src)
            # overflow tokens + weight set 2, emitted AFTER the primary
            # weight DMAs so the shared-tag write order is primary ->
            # overflow. The weight transfers WAIT on each buffer's last
            # primary read (~600us); a dma_start blocks its issuing
            # engine's stream until the wait clears, so they ride the
            # GpSimd SWDGE ring (idle after the warmup memset) -- scalar
            # still owes every silu and sync the yt output writes.
            if R:
                nc.gpsimd.dma_start(wg2_0a[:], wg2[:, 0, :half])
                nc.gpsimd.dma_start(wg2_0b[:], wg2[:, 0, half:])
                for m in range(1, KH):
                    nc.gpsimd.dma_start(wg2_sb[m - 1][:], wg2[:, m])
                nc.gpsimd.dma_start(wu2_0[:], wu2[:, 0])
                for m in range(1, KH):
                    nc.gpsimd.dma_start(wu2_sb[m - 1][:], wu2[:, m])
                for m2 in range(KD):
                    nc.gpsimd.dma_start(wd2_sb[m2][:], wd2[:, m2])

            # Overflow helpers; emitted as a separate region after the
            # primary tiles (interleaving them into tile-7's stream measured
            # slower: 44-col chains have LDWEIGHTS longer than their moving
            # phase and break the dense stream's LDW pipelining).
            if R:
                h2o = {}

                def ovf_gate_up_block(m):
                    h2o[m] = kpool.tile([P, TOK], dt.bfloat16, tag=f"sgk{m}",
                                        name=f"h2o{m}")
                    pg = psA.tile([P, TOK], dt.float32, tag="pg", name=f"pgo_{m}")
                    for k in range(KD):
                        nc.tensor.matmul(
                            pg[:, :R], wg2_at(m, k), xt2_sb[:, k],
                            start=(k == 0), stop=(k == KD - 1),
                        )
                    pu = psA.tile([P, TOK], dt.float32, tag="pu", name=f"puo_{m}")
                    for k in range(KD):
                        nc.tensor.matmul(
                            pu[:, :R], wu2_at(m, k), xt2_sb[:, k],
                            start=(k == 0), stop=(k == KD - 1),
                        )
                    pgs = spool.tile([P, TOK], dt.float32, tag="pgs")
                    nc.vector.tensor_copy(pgs[:, :R], pg[:, :R])
                    pus = spool.tile([P, TOK], dt.float32, tag="pus")
                    nc.vector.tensor_copy(pus[:, :R], pu[:, :R])
                    sgo = spool.tile([P, TOK], dt.bfloat16, tag="sg")
                    nc.scalar.activation(
                        sgo[:, :R], pgs[:, :R],
                        mybir.ActivationFunctionType.Silu,
                    )
                    nc.vector.tensor_mul(h2o[m][:, :R], sgo[:, :R], pus[:, :R])

                def ovf_down_block(m2):
                    py = psB.tile([P, TOK], dt.float32, tag="py", name=f"pyo_{m2}")
                    for k2 in range(KH):
                        nc.tensor.matmul(
                            py[:, :R], wd2_sb[m2][:, k2], h2o[k2][:, :R],
                            start=(k2 == 0), stop=(k2 == KH - 1),
                        )
                    ot = opool.tile([P, TOK], dt.bfloat16, tag="ot")
                    nc.vector.tensor_copy(ot[:, :R], py[:, :R])
                    nc.sync.dma_start(yt[:, m2, B : B + R], ot[:, :R])

            last_t = len(tiles) - 1
            # ovf emission points live in the generic (t_i > 0) branch
            assert not R or last_t >= 1, "overflow needs >= 2 primary tiles"
            off = 0
            for t_i, tok in enumerate(tiles):
                ts = slice(off, off + tok)
                off += tok
                xt_sb = xt_tiles[t_i]
                if t_i == 0:
                    q = KD // 4
                    rhs = lambda k, _p=xt_sb: _p[k // q][:, k % q]
                else:
                    rhs = lambda k, _x=xt_sb: _x[:, k]

                # h2 is one tile for full tiles: the first down matmul then
                # carries a single wait (no hoisted-NoOp slot on the PE
                # queue, which costs a full 216ns each). Only a final ragged
                # tile (R == 0 case) uses per-m tiles, letting its down
                # chains start before the last silu/mul retires (tail trim).
                last = t_i == len(tiles) - 1 and tok < TOK and not R
                if last:
                    h2_sb = [
                        kpool.tile([P, TOK], dt.bfloat16, tag=f"sgk{m}", name=f"h2m{m}")
                        for m in range(KH)
                    ]
                    h2w = lambda m: h2_sb[m][:, :tok]
                    h2r = lambda k2: h2_sb[k2][:, :tok]
                else:
                    h2_sb = hpool.tile([P, KH, TOK], dt.bfloat16, tag="h2",
                                       name=f"h2_{off}")
                    h2w = lambda m: h2_sb[:, m, :tok]
                    h2r = lambda k2: h2_sb[:, k2, :tok]
                if t_i == 0:
                    # first tile: ALL gate chains, then ALL up chains, so the
                    # m=0 up weights are not needed until ~42us into the
                    # kernel - matching what the DMA rings can deliver. The
                    # silu output for each m is kept in a dedicated tile
                    # until the up sweep reaches it.
                    sg_keep = []
                    for m in range(KH):
                        pg = psA.tile([P, TOK], dt.float32, tag="pg", name=f"pg0_{m}")
                        for k in range(KD):
                            nc.tensor.matmul(
                                pg[:, :tok], wg_at(m, k), rhs(k)[:, :tok],
                                start=(k == 0), stop=(k == KD - 1),
                            )
                        pgs = spool.tile([P, TOK], dt.float32, tag="pgs")
                        nc.vector.tensor_copy(pgs[:, :tok], pg[:, :tok])
                        sgm = kpool.tile(
                            [P, TOK], dt.bfloat16, tag=f"sgk{m}", name=f"sgk{m}"
                        )
                        nc.scalar.activation(
                            sgm[:, :tok], pgs[:, :tok],
                            mybir.ActivationFunctionType.Silu,
                        )
                        sg_keep.append(sgm)
                    for m in range(KH):
                        pu = psA.tile([P, TOK], dt.float32, tag="pu", name=f"pu0_{m}")
                        for k in range(KD):
                            nc.tensor.matmul(
                                pu[:, :tok], wu_at(m, k), rhs(k)[:, :tok],
                                start=(k == 0), stop=(k == KD - 1),
                            )
                        pus = spool.tile([P, TOK], dt.float32, tag="pus")
                        nc.vector.tensor_copy(pus[:, :tok], pu[:, :tok])
                        nc.vector.tensor_mul(
                            h2w(m), sg_keep[m][:, :tok], pus[:, :tok]
                        )
                else:
                    for m in range(KH):
                        pg = psA.tile([P, TOK], dt.float32, tag="pg", name=f"pg{off}_{m}")
                        for k in range(KD):
                            nc.tensor.matmul(
                                pg[:, :tok], wg_at(m, k), rhs(k)[:, :tok],
                                start=(k == 0), stop=(k == KD - 1),
                            )
                        pu = psA.tile([P, TOK], dt.float32, tag="pu", name=f"pu{off}_{m}")
                        for k in range(KD):
                            nc.tensor.matmul(
                                pu[:, :tok], wu_at(m, k), rhs(k)[:, :tok],
                                start=(k == 0), stop=(k == KD - 1),
                            )
                        # fast DVE copies release the PSUM banks immediately;
                        # silu+mul then run off SBUF, off the bank-recycle path
                        pgs = spool.tile([P, TOK], dt.float32, tag="pgs")
                        nc.vector.tensor_copy(pgs[:, :tok], pg[:, :tok])
                        pus = spool.tile([P, TOK], dt.float32, tag="pus")
                        nc.vector.tensor_copy(pus[:, :tok], pu[:, :tok])
                        sg = spool.tile([P, TOK], dt.bfloat16, tag="sg")
                        nc.scalar.activation(
                            sg[:, :tok], pgs[:, :tok],
                            mybir.ActivationFunctionType.Silu,
                        )
                        nc.vector.tensor_mul(
                            h2w(m), sg[:, :tok], pus[:, :tok]
                        )

                for m2 in range(KD):
                    py = psB.tile([P, TOK], dt.float32, tag="py", name=f"py{off}_{m2}")
                    for k2 in range(KH):
                        nc.tensor.matmul(
                            py[:, :tok], wd_sb[m2][:, k2], h2r(k2),
                            start=(k2 == 0), stop=(k2 == KH - 1),
                        )
                    ot = opool.tile([P, TOK], dt.bfloat16, tag="ot")
                    nc.vector.tensor_copy(ot[:, :tok], py[:, :tok])
                    nc.sync.dma_start(yt[:, m2, ts], ot[:, :tok])

            # ---------------- overflow tile (weight set 2) ----------------
            if R:
                for m in range(KH):
                    ovf_gate_up_block(m)
                for m2 in range(KD):
                    ovf_down_block(m2)

    return nc


def pack_lhsT(w: np.ndarray) -> np.ndarray:
    """[K, M] weight -> [p=128, m_block, k_chunk, 128] bf16, so that
    slice [:, m, k, :] is the lhsT tile for contraction chunk k, output
    block m, and each [:, m] block is one contiguous DMA."""
    K, M = w.shape
    kc, mb = K // P, M // P
    return np.ascontiguousarray(
        w.reshape(kc, P, mb, P).transpose(1, 2, 0, 3)
    ).astype(_BF16)


def pack_tokens(xe: np.ndarray, C: int) -> np.ndarray:
    """[n, D] tokens -> zero-padded [p=128, C*KD] bf16, blocked per token
    tile as [KD, tok] per partition (one contiguous DMA per tile)."""
    n = xe.shape[0]
    out = np.zeros((P, C * KD), dtype=_BF16)
    off = 0
    for tok in token_tiles(C):
        xe_t = xe[off : min(off + tok, n)]
        nt = xe_t.shape[0]
        if nt:
            blk = np.zeros((P, KD, tok), dtype=_BF16)
            # [nt, D] -> [D, nt] -> [KD, P, nt] -> [P, KD, nt]
            blk[:, :, :nt] = (
                xe_t.T.reshape(KD, P, nt).transpose(1, 0, 2).astype(_BF16)
            )
            out[:, off * KD : (off + tok) * KD] = blk.reshape(P, KD * tok)
        off += tok
    return out


def route_tokens(xf: np.ndarray, router_w: np.ndarray):
    """Top-2 routing identical to the reference (softmax over selected)."""
    logits = xf @ router_w  # [T, E]
    # top-2 per token (order irrelevant: softmax over the pair + scatter)
    top_idx = np.argpartition(-logits, TOP_K, axis=-1)[:, :TOP_K]
    tv = np.take_along_axis(logits, top_idx, axis=-1)
    tv = tv - tv.max(axis=-1, keepdims=True)
    ev = np.exp(tv)
    probs = ev / ev.sum(axis=-1, keepdims=True)

    idx, scale = [], []
    for e in range(NUM_EXPERTS):
        hit = top_idx == e  # [T, 2]
        rows = np.nonzero(hit.any(axis=-1))[0]
        w = np.where(hit[rows, 0], probs[rows, 0], probs[rows, 1])
        idx.append(rows)
        scale.append(w.astype(np.float32))
    return idx, scale


def plan_overflow(idx):
    """Split each expert's tokens beyond B_PRIMARY into chunks of at most
    R (minimal R with at most NUM_EXPERTS chunks) and assign one chunk
    per core. Returns (B, R, chunks) with chunks[c] = (expert, lo, hi)
    slicing idx[expert][lo:hi], or None."""
    n_e = [len(r) for r in idx]
    if max(n_e) <= B_PRIMARY:
        return min(B_PRIMARY, max(n_e)), 0, [None] * NUM_EXPERTS
    B = B_PRIMARY
    o = [max(n - B, 0) for n in n_e]
    R = None
    for r in range(1, max(o) + 1):
        if sum(-(-oe // r) for oe in o if oe) <= NUM_EXPERTS:
            R = r
            break
    chunks = []
    for e in range(NUM_EXPERTS):
        lo = B
        while lo < n_e[e]:
            hi = min(lo + R, n_e[e])
            chunks.append((e, lo, hi))
            lo = hi
    assert len(chunks) <= NUM_EXPERTS
    chunks += [None] * (NUM_EXPERTS - len(chunks))
    return B, R, chunks


def prepare_in_maps(x, router_w, w_gate, w_up, w_down):
    x = np.asarray(x, dtype=np.float32)
    xf = x.reshape(-1, EMB)
    idx, scale = route_tokens(xf, np.asarray(router_w, dtype=np.float32))
    B, R, chunks = plan_overflow(idx)

    wg_all = np.asarray(w_gate, dtype=np.float32)
    wu_all = np.asarray(w_up, dtype=np.float32)
    wd_all = np.asarray(w_down, dtype=np.float32)
    packed = [
        (pack_lhsT(wg_all[e]), pack_lhsT(wu_all[e]), pack_lhsT(wd_all[e]))
        for e in range(NUM_EXPERTS)
    ]

    in_maps = []
    for c in range(NUM_EXPERTS):
        im = {
            "xt": pack_tokens(xf[idx[c][:B]], B),
            "wg": packed[c][0],
            "wu": packed[c][1],
            "wd": packed[c][2],
        }
        if R:
            ch = chunks[c]
            e2 = ch[0] if ch else c
            rows = idx[e2][ch[1]:ch[2]] if ch else np.zeros(0, dtype=np.int64)
            im["xt2"] = pack_tokens(xf[rows], R)
            im["wg2"] = packed[e2][0]
            im["wu2"] = packed[e2][1]
            im["wd2"] = packed[e2][2]
        in_maps.append(im)
    return in_maps, idx, scale, (B, R), xf


def kernel(x, router_w, w_gate, w_up, w_down):
    from concourse.bass_utils import run_bass_kernel_spmd

    in_maps, idx, scale, cfg, xf = prepare_in_maps(
        x, router_w, w_gate, w_up, w_down
    )
    B, R = cfg
    _, _, chunks = plan_overflow(idx)
    nc = build_moe_expert_kernel(cfg)
    res = None
    last_exc = None
    for _attempt in range(3):
        try:
            res = run_bass_kernel_spmd(nc, in_maps, list(range(NUM_EXPERTS)))
            break
        except Exception as exc:  # transient device wedge: retry
            last_exc = exc
    if res is None:
        raise last_exc

    C = B + R
    out = np.zeros_like(xf)
    for c in range(NUM_EXPERTS):
        ytc = np.asarray(res.results[c]["yt"]).astype(np.float32)  # [P, KD, C]
        y = ytc.transpose(1, 0, 2).reshape(EMB, C)  # [D, C]
        nb = min(len(idx[c]), B)
        # indices within one scatter are unique -> fancy += is safe;
        # primary and overflow scatters are done separately because a
        # token may appear in both (different experts).
        out[idx[c][:nb]] += y[:, :nb].T * scale[c][:nb, None]
        if R and chunks[c]:
            e2, lo, hi = chunks[c]
            out[idx[e2][lo:hi]] += y[:, B : B + hi - lo].T * scale[e2][lo:hi, None]
    return out.reshape(np.asarray(x).shape)
